# revision 21
# baseline (speedup 1.0000x reference)
"""Fused GEMM + bias + residual + AvgPool2d(2) + global-mean normalize, 8-core SPMD.

Reference computation (B=8192, IN_F=1024, OUT_F=4096, S=64, K=2):
    out_lin = x @ W.T + bias + y                  # (B, 4096)
    pooled  = avgpool2x2(out_lin.reshape(B,64,64))# (B, 32, 32)
    out     = pooled / pooled.mean()              # (B, 1, 32, 32)

Key algebraic folds used by the kernel (all exact):
  * The 2x2 avg-pool is linear, so it folds into the weight/bias/residual:
        pooled_raw[b, m] = x[b] . Wsum[m] + bias_sum[m] + y_sum[b, m]
    where m = 32*i + j pools OUT_F rows {128i+2j, 128i+2j+1, 128i+64+2j,
    128i+64+2j+1}, and Wsum/bias_sum/y_sum are 4-row/element sums.
    This shrinks the GEMM N-dim 4096 -> 1024 (4x fewer FLOPs) and never
    materializes the (B, 4096) intermediate.
  * The 1/4 pool factor cancels between numerator and global mean:
        out = pooled_raw * (B*1024 / sum_global(pooled_raw))
  * The global sum also decomposes over raw inputs:
        local_sum = xsum . wcolsum + BL * bias_tot + ytot
    so the one scalar AllReduce fires as soon as the inputs are reduced,
    overlapping its latency (and cross-core skew) with the GEMM tail.

Performance notes:
  * GEMM inputs cast to bf16 on-chip (fp32 PSUM accumulation); pooling sums
    and the output stay fp32.  Scale-relative error ~1.6e-3.
  * DMA needs >=8KB descriptors and a >=16-wide outer iteration dim per DMA
    (descriptors are engine-assigned by outer index) to reach ~370 GB/s.
    W row-pairs (2j, 2j+1) are contiguous, so W is loaded as 16 x 1 MiB DMAs
    iterated j-major; the resulting partition permutation p = 4j + a is
    undone for free inside the PE-transpose PSUM->SBUF copy.
  * Streams are kept on separate engine FIFOs to avoid head-of-line
    coupling: rings (sync+scalar) trigger DMAs, DVE does the W/y pooling
    adds and epilogue, ACT does the PSUM->SBUF transpose copies, gpsimd
    pools half the y tiles.  W is front-loaded on the rings; y/x/GEMM
    chase it b-tile by b-tile.

Sharding: batch B split 8 ways (1024 rows/core); weight + bias replicated.
"""

import numpy as np

import concourse.bass as bass
import concourse.mybir as mybir
import concourse.tile as tile
from concourse import bacc
from concourse.bass import ts
from concourse.bass_utils import run_bass_kernel_spmd
from concourse.masks import make_identity

N_CORES = 8
B = 8192
BL = B // N_CORES          # 1024 batch rows per core
KF = 1024                  # IN_F (contraction)
NF = 4096                  # OUT_F
M = 1024                   # pooled features (32*32)
TOT = float(B * M)         # elements in the global mean
F32 = mybir.dt.float32
BF16 = mybir.dt.bfloat16
ADD = mybir.AluOpType.add
MULT = mybir.AluOpType.mult

_CACHE = {}


def build_nc():
    nc = bacc.Bacc("TRN2", target_bir_lowering=False, debug=False,
                   num_devices=N_CORES)
    x = nc.dram_tensor("x", [BL, KF], F32, kind="ExternalInput").ap()
    y = nc.dram_tensor("y", [BL, NF], F32, kind="ExternalInput").ap()
    w = nc.dram_tensor("w", [NF, KF], F32, kind="ExternalInput").ap()
    b = nc.dram_tensor("b", [1, NF], F32, kind="ExternalInput").ap()
    out = nc.dram_tensor("out", [BL, M], F32, kind="ExternalOutput").ap()

    # W row n = 512g + 128a + 64r + 2j + s; pooled feature m = 128g + 32a + j;
    # (r, s) are the 4 pooled taps.  Row-pair index np = 256g + 64a + 32r + j.
    w_pairs = w.rearrange("(n s) k -> n (s k)", s=2)          # [2048, 2048]
    wv = w_pairs.rearrange("(g a r j) kk -> g r j a kk", a=4, r=2, j=32)
    bview = b.rearrange("o (i r j s) -> o i r j s", r=2, j=32, s=2)

    ring = [nc.sync, nc.scalar]

    with tile.TileContext(nc) as tc:
        with (
            tc.tile_pool(name="consts", bufs=1) as consts,
            tc.tile_pool(name="wload", bufs=3) as wload,
            tc.tile_pool(name="wsump", bufs=1) as wsump,
            tc.tile_pool(name="wtp", bufs=1) as wtp,
            tc.tile_pool(name="xload", bufs=3) as xload,
            tc.tile_pool(name="xtp", bufs=2) as xtp,
            tc.tile_pool(name="yload", bufs=4) as yload,
            tc.tile_pool(name="yup", bufs=3) as yup,
            tc.tile_pool(name="ysump", bufs=1) as ysump,
            tc.tile_pool(name="statsp", bufs=1) as statsp,
            tc.tile_pool(name="outp", bufs=2) as outp,
            tc.tile_pool(name="psA", bufs=4, space="PSUM") as psA,
            tc.tile_pool(name="psT", bufs=3, space="PSUM") as psT,
            tc.tile_pool(name="psB", bufs=1, space="PSUM") as psB,
            tc.tile_pool(name="dram", bufs=1, space="DRAM") as dram,
        ):
            # ---- constants ----
            ident = consts.tile([128, 128], BF16)
            make_identity(nc, ident)
            ident_f = consts.tile([128, 128], F32)
            make_identity(nc, ident_f)
            ones_row = consts.tile([1, 128], BF16)
            nc.vector.memset(ones_row, 1.0)
            ones_col = consts.tile([128, 1], F32)
            nc.vector.memset(ones_col, 1.0)

            # ---- bias: one contiguous load (borrows a W-pool slot),
            # then pool 4096 -> 1024 with three 1-partition DVE adds ----
            bload = wload.tile([1, NF], F32, tag="wl", name="bload")
            nc.sync.dma_start(out=bload, in_=b)
            blv = bload.rearrange("o (i r j s) -> o i r j s", r=2, j=32, s=2)
            bsum = consts.tile([1, 32, 32], F32)
            nc.vector.tensor_add(bsum, blv[:, :, 0, :, 0], blv[:, :, 0, :, 1])
            nc.vector.tensor_add(bsum, bsum, blv[:, :, 1, :, 0])
            nc.vector.tensor_add(bsum, bsum, blv[:, :, 1, :, 1])
            bsum_bf = consts.tile([1, M], BF16)
            nc.vector.tensor_copy(out=bsum_bf,
                                  in_=bsum.rearrange("o i j -> o (i j)"))
            btot = consts.tile([1, 1], F32)
            nc.vector.reduce_sum(out=btot,
                                 in_=bsum.rearrange("o i j -> o (i j)"),
                                 axis=mybir.AxisListType.X)
            btot_s = consts.tile([1, 1], F32)
            nc.scalar.mul(btot_s, btot, float(BL))
            ones_one = consts.tile([1, 1], F32)
            nc.vector.memset(ones_one, 1.0)

            # ---- W first: rings front-load it; pool rows (bf16), transpose
            # to [k, m].  DVE does only the adds; ACT does the PSUM copies.
            wt_all = wtp.tile([128, 8, M], BF16)
            for g in range(8):
                wl = wload.tile([128, 2, 2048], F32)
                eng = ring[(g + 1) % 2]
                for r in range(2):
                    eng.dma_start(out=wl[:, r, :], in_=wv[g, r])
                wlv = wl.rearrange("p r (s k) -> p r s k", s=2)
                t1 = wsump.tile([128, KF], F32)
                nc.vector.tensor_add(t1, wlv[:, 0, 0], wlv[:, 0, 1])
                t2 = wsump.tile([128, KF], F32)
                nc.vector.tensor_add(t2, wlv[:, 1, 0], wlv[:, 1, 1])
                wsum = wsump.tile([128, KF], BF16, bufs=2)
                nc.vector.tensor_add(wsum, t1, t2)
                for kb in range(8):
                    pt = psT.tile([128, 128], BF16, tag="pt",
                                  name=f"ptw{g}_{kb}")
                    nc.tensor.transpose(pt, wsum[:, ts(kb, 128)], ident)
                    # undo the j-major load permutation (psum col p = 4j + a
                    # -> wt col 32a + j)
                    nc.scalar.copy(
                        out=wt_all[:, kb, ts(g, 128)].rearrange(
                            "k (a j) -> k j a", a=4),
                        in_=pt.rearrange("k (j a) -> k j a", a=4))

            # wcolsum[k] = sum_m Wsum[m, k], reduced from bf16 wt (free dim)
            wcol_r = statsp.tile([128, 8, 1], F32)
            nc.vector.reduce_sum(out=wcol_r, in_=wt_all,
                                 axis=mybir.AxisListType.X)

            # ---- stream y + x per b-tile; transpose x; GEMM; epilogue ----
            combo = statsp.tile([128, 16], F32)
            psums_all = combo[:, 8:16]
            xsum_acc = statsp.tile([128, 8], F32)
            ys_tiles = {}
            for bt in range(8):
                veng = nc.vector if bt % 2 == 0 else nc.gpsimd
                ys = ysump.tile([128, M], F32, tag=f"ys{bt}", name=f"ys{bt}")
                for nh in range(2):
                    yt = yload.tile([128, 2048], F32)
                    ring[bt % 2].dma_start(out=yt,
                                           in_=y[ts(bt, 128), ts(nh, 2048)])
                    ytv = yt.rearrange("p (q s) -> p q s", s=2)
                    u = yup.tile([128, KF], F32)
                    veng.tensor_add(u, ytv[:, :, 0], ytv[:, :, 1])
                    u2 = u.rearrange("p (i r j) -> p i r j", r=2, j=32)
                    veng.tensor_add(
                        ys[:, ts(nh, 512)].rearrange("p (i j) -> p i j", j=32),
                        u2[:, :, 0, :], u2[:, :, 1, :])
                nc.vector.reduce_sum(out=psums_all[:, bt:bt + 1], in_=ys,
                                     axis=mybir.AxisListType.X)
                ys_tiles[bt] = ys

                xf = xload.tile([128, KF], F32)
                ring[(bt + 1) % 2].dma_start(out=xf, in_=x[ts(bt, 128), :])
                xT = xtp.tile([128, 8, 128], BF16, tag="xT", name=f"xT{bt}")
                for kb in range(8):
                    pt = psT.tile([128, 128], F32, tag="pt",
                                  name=f"ptx{bt}_{kb}")
                    nc.tensor.transpose(pt, xf[:, ts(kb, 128)], ident_f)
                    nc.vector.tensor_copy(out=xT[:, kb, :], in_=pt)
                # xsum[k] += sum_b x[b, k] (from the bf16 transposed copy)
                xs_r = statsp.tile([128, 8, 1], F32, tag="xs_r", bufs=2,
                                   name=f"xs_r{bt}")
                nc.vector.reduce_sum(out=xs_r, in_=xT,
                                     axis=mybir.AxisListType.X)
                if bt == 0:
                    nc.vector.tensor_copy(out=xsum_acc, in_=xs_r[:, :, 0])
                else:
                    nc.vector.tensor_add(xsum_acc, xsum_acc, xs_r[:, :, 0])

                mm = [psA.tile([128, 512], F32, tag="mm", name=f"mm{bt}_{h}")
                      for h in range(2)]
                for kb in range(8):
                    for mh in range(2):
                        nc.tensor.matmul(mm[mh], xT[:, kb, :],
                                         wt_all[:, kb, ts(mh, 512)],
                                         start=(kb == 0), stop=False)
                for mh in range(2):
                    nc.tensor.matmul(mm[mh], ones_row, bsum_bf[:, ts(mh, 512)],
                                     start=False, stop=True)
                    nc.vector.tensor_add(ys[:, ts(mh, 512)], mm[mh],
                                         ys[:, ts(mh, 512)])

            # ---- local sum -> AllReduce (overlaps the GEMM tail) ----
            # local_sum = xsum . wcolsum + BL * bias_tot + ytot
            nc.vector.tensor_mul(combo[:, 0:8], xsum_acc, wcol_r[:, :, 0])
            part = statsp.tile([128, 1], F32)
            nc.vector.reduce_sum(out=part, in_=combo,
                                 axis=mybir.AxisListType.X)
            ls_ps = psB.tile([1, 1], F32, tag="small", name="ls_ps")
            nc.tensor.matmul(ls_ps, part, ones_col, start=True, stop=False)
            nc.tensor.matmul(ls_ps, btot_s, ones_one, start=False, stop=True)
            ls2 = statsp.tile([1, 1], F32)
            nc.vector.tensor_copy(out=ls2, in_=ls_ps)

            cc_in = dram.tile([1, 1], F32)
            cc_out = dram.tile([1, 1], F32)
            nc.sync.dma_start(out=cc_in, in_=ls2)
            nc.gpsimd.collective_compute(
                "AllReduce", ADD,
                replica_groups=[list(range(N_CORES))],
                ins=[cc_in.opt()], outs=[cc_out.opt()])
            # broadcast gsum to all partitions in the readback DMA
            gsb = statsp.tile([128, 1], F32)
            nc.sync.dma_start(out=gsb, in_=cc_out.to_broadcast((128, 1)))
            rsb = statsp.tile([128, 1], F32)
            nc.vector.reciprocal(rsb, gsb)

            # ---- normalize + store: out = pooled * (1/gsum) * TOT ----
            for bt in range(8):
                meng = nc.vector if bt % 2 == 0 else nc.gpsimd
                ot = outp.tile([128, M], F32)
                meng.tensor_scalar(out=ot, in0=ys_tiles[bt],
                                   scalar1=rsb, scalar2=TOT,
                                   op0=MULT, op1=MULT)
                ring[bt % 2].dma_start(out=out[ts(bt, 128), :], in_=ot)

    nc.compile()
    return nc


def _run(inputs, trace=False):
    if "nc" not in _CACHE:
        _CACHE["nc"] = build_nc()
    nc = _CACHE["nc"]
    x = np.ascontiguousarray(np.asarray(inputs["x"], dtype=np.float32))
    y = np.ascontiguousarray(np.asarray(inputs["y"], dtype=np.float32))
    w = np.ascontiguousarray(np.asarray(inputs["weight"], dtype=np.float32))
    b = np.ascontiguousarray(
        np.asarray(inputs["bias"], dtype=np.float32).reshape(1, NF))
    in_maps = [
        {"x": x[c * BL:(c + 1) * BL], "y": y[c * BL:(c + 1) * BL],
         "w": w, "b": b}
        for c in range(N_CORES)
    ]
    res = run_bass_kernel_spmd(nc, in_maps, core_ids=list(range(N_CORES)),
                               trace=trace)
    full = np.concatenate([res.results[c]["out"] for c in range(N_CORES)],
                          axis=0)
    return full.reshape(B, 1, 32, 32), res


def kernel(**inputs) -> np.ndarray:
    out, _ = _run(inputs, trace=False)
    return out


# revision 22
# speedup vs baseline: 1.0006x; 1.0006x over previous
"""Fused GEMM + bias + residual + AvgPool2d(2) + global-mean normalize, 8-core SPMD.

Reference computation (B=8192, IN_F=1024, OUT_F=4096, S=64, K=2):
    out_lin = x @ W.T + bias + y                  # (B, 4096)
    pooled  = avgpool2x2(out_lin.reshape(B,64,64))# (B, 32, 32)
    out     = pooled / pooled.mean()              # (B, 1, 32, 32)

Key algebraic folds used by the kernel (all exact):
  * The 2x2 avg-pool is linear, so it folds into the weight/bias/residual:
        pooled_raw[b, m] = x[b] . Wsum[m] + bias_sum[m] + y_sum[b, m]
    where m = 32*i + j pools OUT_F rows {128i+2j, 128i+2j+1, 128i+64+2j,
    128i+64+2j+1}, and Wsum/bias_sum/y_sum are 4-row/element sums.
    This shrinks the GEMM N-dim 4096 -> 1024 (4x fewer FLOPs) and never
    materializes the (B, 4096) intermediate.
  * The 1/4 pool factor cancels between numerator and global mean:
        out = pooled_raw * (B*1024 / sum_global(pooled_raw))
  * The global sum also decomposes over raw inputs:
        local_sum = xsum . wcolsum + BL * bias_tot + ytot
    so the one scalar AllReduce fires as soon as the inputs are reduced,
    overlapping its latency (and cross-core skew) with the GEMM tail.

Performance notes:
  * GEMM inputs cast to bf16 on-chip (fp32 PSUM accumulation); pooling sums
    and the output stay fp32.  Scale-relative error ~1.6e-3.
  * DMA needs >=8KB descriptors and a >=16-wide outer iteration dim per DMA
    (descriptors are engine-assigned by outer index) to reach ~370 GB/s.
    W row-pairs (2j, 2j+1) are contiguous, so W is loaded as 16 x 1 MiB DMAs
    iterated j-major; the resulting partition permutation p = 4j + a is
    undone for free inside the PE-transpose PSUM->SBUF copy.
  * Streams are kept on separate engine FIFOs to avoid head-of-line
    coupling: rings (sync+scalar) trigger DMAs, DVE does the W/y pooling
    adds and epilogue, ACT does the PSUM->SBUF transpose copies, gpsimd
    pools half the y tiles.  W is front-loaded on the rings; y/x/GEMM
    chase it b-tile by b-tile.

Sharding: batch B split 8 ways (1024 rows/core); weight + bias replicated.
"""

import numpy as np

import concourse.bass as bass
import concourse.mybir as mybir
import concourse.tile as tile
from concourse import bacc
from concourse.bass import ts
from concourse.bass_utils import run_bass_kernel_spmd
from concourse.masks import make_identity

N_CORES = 8
B = 8192
BL = B // N_CORES          # 1024 batch rows per core
KF = 1024                  # IN_F (contraction)
NF = 4096                  # OUT_F
M = 1024                   # pooled features (32*32)
TOT = float(B * M)         # elements in the global mean
F32 = mybir.dt.float32
BF16 = mybir.dt.bfloat16
ADD = mybir.AluOpType.add
MULT = mybir.AluOpType.mult

_CACHE = {}


def build_nc():
    nc = bacc.Bacc("TRN2", target_bir_lowering=False, debug=False,
                   num_devices=N_CORES)
    x = nc.dram_tensor("x", [BL, KF], F32, kind="ExternalInput").ap()
    y = nc.dram_tensor("y", [BL, NF], F32, kind="ExternalInput").ap()
    w = nc.dram_tensor("w", [NF, KF], F32, kind="ExternalInput").ap()
    b = nc.dram_tensor("b", [1, NF], F32, kind="ExternalInput").ap()
    out = nc.dram_tensor("out", [BL, M], F32, kind="ExternalOutput").ap()

    # W row n = 512g + 128a + 64r + 2j + s; pooled feature m = 128g + 32a + j;
    # (r, s) are the 4 pooled taps.  Row-pair index np = 256g + 64a + 32r + j.
    w_pairs = w.rearrange("(n s) k -> n (s k)", s=2)          # [2048, 2048]
    wv = w_pairs.rearrange("(g a r j) kk -> g r j a kk", a=4, r=2, j=32)

    ring = [nc.sync, nc.scalar]

    with tile.TileContext(nc) as tc:
        with (
            tc.tile_pool(name="consts", bufs=1) as consts,
            tc.tile_pool(name="wload", bufs=3) as wload,
            tc.tile_pool(name="wsump", bufs=1) as wsump,
            tc.tile_pool(name="wtp", bufs=1) as wtp,
            tc.tile_pool(name="xload", bufs=3) as xload,
            tc.tile_pool(name="xtp", bufs=2) as xtp,
            tc.tile_pool(name="yload", bufs=4) as yload,
            tc.tile_pool(name="yup", bufs=3) as yup,
            tc.tile_pool(name="ysump", bufs=1) as ysump,
            tc.tile_pool(name="statsp", bufs=1) as statsp,
            tc.tile_pool(name="outp", bufs=2) as outp,
            tc.tile_pool(name="psA", bufs=4, space="PSUM") as psA,
            tc.tile_pool(name="psT", bufs=3, space="PSUM") as psT,
            tc.tile_pool(name="psB", bufs=1, space="PSUM") as psB,
            tc.tile_pool(name="dram", bufs=1, space="DRAM") as dram,
        ):
            # ---- constants ----
            ident = consts.tile([128, 128], BF16)
            make_identity(nc, ident)
            ident_f = consts.tile([128, 128], F32)
            make_identity(nc, ident_f)
            ones_row = consts.tile([1, 128], BF16)
            nc.vector.memset(ones_row, 1.0)
            ones_col = consts.tile([128, 1], F32)
            nc.vector.memset(ones_col, 1.0)

            # ---- bias: one contiguous load (borrows a W-pool slot),
            # then pool 4096 -> 1024 with three 1-partition DVE adds ----
            bload = wload.tile([1, NF], F32, tag="wl", name="bload")
            nc.sync.dma_start(out=bload, in_=b)
            blv = bload.rearrange("o (i r j s) -> o i r j s", r=2, j=32, s=2)
            bsum = consts.tile([1, 32, 32], F32)
            nc.vector.tensor_add(bsum, blv[:, :, 0, :, 0], blv[:, :, 0, :, 1])
            nc.vector.tensor_add(bsum, bsum, blv[:, :, 1, :, 0])
            nc.vector.tensor_add(bsum, bsum, blv[:, :, 1, :, 1])
            bsum_bf = consts.tile([1, M], BF16)
            nc.vector.tensor_copy(out=bsum_bf,
                                  in_=bsum.rearrange("o i j -> o (i j)"))
            btot = consts.tile([1, 1], F32)
            nc.vector.reduce_sum(out=btot,
                                 in_=bsum.rearrange("o i j -> o (i j)"),
                                 axis=mybir.AxisListType.X)
            btot_s = consts.tile([1, 1], F32)
            nc.scalar.mul(btot_s, btot, float(BL))
            ones_one = consts.tile([1, 1], F32)
            nc.vector.memset(ones_one, 1.0)

            # ---- W first: rings front-load it; pool rows (bf16), transpose
            # to [k, m].  DVE does only the adds; ACT does the PSUM copies.
            wt_all = wtp.tile([128, 8, M], BF16)
            for g in range(8):
                wl = wload.tile([128, 2, 2048], F32)
                eng = ring[(g + 1) % 2]
                for r in range(2):
                    eng.dma_start(out=wl[:, r, :], in_=wv[g, r])
                wlv = wl.rearrange("p r (s k) -> p r s k", s=2)
                t1 = wsump.tile([128, KF], F32)
                nc.vector.tensor_add(t1, wlv[:, 0, 0], wlv[:, 0, 1])
                t2 = wsump.tile([128, KF], F32)
                nc.vector.tensor_add(t2, wlv[:, 1, 0], wlv[:, 1, 1])
                wsum = wsump.tile([128, KF], BF16, bufs=2)
                nc.vector.tensor_add(wsum, t1, t2)
                for kb in range(8):
                    pt = psT.tile([128, 128], BF16, tag="pt",
                                  name=f"ptw{g}_{kb}")
                    nc.tensor.transpose(pt, wsum[:, ts(kb, 128)], ident)
                    # undo the j-major load permutation (psum col p = 4j + a
                    # -> wt col 32a + j)
                    nc.scalar.copy(
                        out=wt_all[:, kb, ts(g, 128)].rearrange(
                            "k (a j) -> k j a", a=4),
                        in_=pt.rearrange("k (j a) -> k j a", a=4))

            # wcolsum[k] = sum_m Wsum[m, k], reduced from bf16 wt (free dim)
            wcol_r = statsp.tile([128, 8, 1], F32)
            nc.vector.reduce_sum(out=wcol_r, in_=wt_all,
                                 axis=mybir.AxisListType.X)

            # ---- stream y + x per b-tile; transpose x; GEMM; epilogue ----
            combo = statsp.tile([128, 16], F32)
            psums_all = combo[:, 8:16]
            xsum_acc = statsp.tile([128, 8], F32)
            ys_tiles = {}
            for bt in range(8):
                veng = nc.vector if bt % 2 == 0 else nc.gpsimd
                ys = ysump.tile([128, M], F32, tag=f"ys{bt}", name=f"ys{bt}")
                for nh in range(2):
                    yt = yload.tile([128, 2048], F32)
                    ring[bt % 2].dma_start(out=yt,
                                           in_=y[ts(bt, 128), ts(nh, 2048)])
                    ytv = yt.rearrange("p (q s) -> p q s", s=2)
                    u = yup.tile([128, KF], F32)
                    veng.tensor_add(u, ytv[:, :, 0], ytv[:, :, 1])
                    u2 = u.rearrange("p (i r j) -> p i r j", r=2, j=32)
                    veng.tensor_add(
                        ys[:, ts(nh, 512)].rearrange("p (i j) -> p i j", j=32),
                        u2[:, :, 0, :], u2[:, :, 1, :])
                nc.vector.reduce_sum(out=psums_all[:, bt:bt + 1], in_=ys,
                                     axis=mybir.AxisListType.X)
                ys_tiles[bt] = ys

                xf = xload.tile([128, KF], F32)
                ring[(bt + 1) % 2].dma_start(out=xf, in_=x[ts(bt, 128), :])
                xT = xtp.tile([128, 8, 128], BF16, tag="xT", name=f"xT{bt}")
                for kb in range(8):
                    pt = psT.tile([128, 128], F32, tag="pt",
                                  name=f"ptx{bt}_{kb}")
                    nc.tensor.transpose(pt, xf[:, ts(kb, 128)], ident_f)
                    nc.vector.tensor_copy(out=xT[:, kb, :], in_=pt)
                # xsum[k] += sum_b x[b, k] (from the bf16 transposed copy)
                xs_r = statsp.tile([128, 8, 1], F32, tag="xs_r", bufs=2,
                                   name=f"xs_r{bt}")
                nc.vector.reduce_sum(out=xs_r, in_=xT,
                                     axis=mybir.AxisListType.X)
                if bt == 0:
                    nc.vector.tensor_copy(out=xsum_acc, in_=xs_r[:, :, 0])
                else:
                    nc.vector.tensor_add(xsum_acc, xsum_acc, xs_r[:, :, 0])

                mm = [psA.tile([128, 512], F32, tag="mm", name=f"mm{bt}_{h}")
                      for h in range(2)]
                for kb in range(8):
                    for mh in range(2):
                        nc.tensor.matmul(mm[mh], xT[:, kb, :],
                                         wt_all[:, kb, ts(mh, 512)],
                                         start=(kb == 0), stop=False)
                for mh in range(2):
                    nc.tensor.matmul(mm[mh], ones_row, bsum_bf[:, ts(mh, 512)],
                                     start=False, stop=True)
                    nc.vector.tensor_add(ys[:, ts(mh, 512)], mm[mh],
                                         ys[:, ts(mh, 512)])

            # ---- local sum -> AllReduce (overlaps the GEMM tail) ----
            # local_sum = xsum . wcolsum + BL * bias_tot + ytot
            nc.vector.tensor_mul(combo[:, 0:8], xsum_acc, wcol_r[:, :, 0])
            part = statsp.tile([128, 1], F32)
            nc.vector.reduce_sum(out=part, in_=combo,
                                 axis=mybir.AxisListType.X)
            ls_ps = psB.tile([1, 1], F32, tag="small", name="ls_ps")
            nc.tensor.matmul(ls_ps, part, ones_col, start=True, stop=False)
            nc.tensor.matmul(ls_ps, btot_s, ones_one, start=False, stop=True)
            ls2 = statsp.tile([1, 1], F32)
            nc.vector.tensor_copy(out=ls2, in_=ls_ps)

            cc_in = dram.tile([1, 1], F32)
            cc_out = dram.tile([1, 1], F32)
            nc.sync.dma_start(out=cc_in, in_=ls2)
            nc.gpsimd.collective_compute(
                "AllReduce", ADD,
                replica_groups=[list(range(N_CORES))],
                ins=[cc_in.opt()], outs=[cc_out.opt()])
            # broadcast gsum to all partitions in the readback DMA
            gsb = statsp.tile([128, 1], F32)
            nc.sync.dma_start(out=gsb, in_=cc_out.to_broadcast((128, 1)))
            rsb = statsp.tile([128, 1], F32)
            nc.vector.reciprocal(rsb, gsb)

            # ---- normalize + store: out = pooled * (1/gsum) * TOT ----
            for bt in range(8):
                meng = nc.vector if bt % 2 == 0 else nc.gpsimd
                ot = outp.tile([128, M], F32)
                meng.tensor_scalar(out=ot, in0=ys_tiles[bt],
                                   scalar1=rsb, scalar2=TOT,
                                   op0=MULT, op1=MULT)
                ring[bt % 2].dma_start(out=out[ts(bt, 128), :], in_=ot)

    nc.compile()
    return nc


def _run(inputs, trace=False):
    if "nc" not in _CACHE:
        _CACHE["nc"] = build_nc()
    nc = _CACHE["nc"]
    x = np.ascontiguousarray(np.asarray(inputs["x"], dtype=np.float32))
    y = np.ascontiguousarray(np.asarray(inputs["y"], dtype=np.float32))
    w = np.ascontiguousarray(np.asarray(inputs["weight"], dtype=np.float32))
    b = np.ascontiguousarray(
        np.asarray(inputs["bias"], dtype=np.float32).reshape(1, NF))
    in_maps = [
        {"x": x[c * BL:(c + 1) * BL], "y": y[c * BL:(c + 1) * BL],
         "w": w, "b": b}
        for c in range(N_CORES)
    ]
    res = run_bass_kernel_spmd(nc, in_maps, core_ids=list(range(N_CORES)),
                               trace=trace)
    full = np.concatenate([res.results[c]["out"] for c in range(N_CORES)],
                          axis=0)
    return full.reshape(B, 1, 32, 32), res


def kernel(**inputs) -> np.ndarray:
    out, _ = _run(inputs, trace=False)
    return out
